# revision 1
# baseline (speedup 1.0000x reference)
"""GTU (gated Toeplitz unit) Bass kernel for 8 TRN2 NeuronCores.

Sharding: tensor-parallel over heads (H=8 -> 1 head/core). Each core
computes its head's u/v projections, the RPE-MLP Toeplitz coefficients,
the causal depthwise long-conv via dense real-DFT matmuls (circular conv
of length 2n realized as TensorE matmuls with constant DFT matrices),
the gate, and a partial o-projection. Host sums the 8 partials + o_b.
"""

import numpy as np

B, N, E = 4, 2048, 1024
H = 8
D1 = 3 * E
DH = D1 // H            # 384
R = 512
GAMMA = 0.99
EPS = 1e-8
M2 = 2 * N              # 4096 (circular conv length)
KH = M2 // 2 + 1        # 2049 rfft bins
KP = 2176               # bins padded to 17*128
KA = 1024 + 128         # augmented contraction for x (bias row), 9*128
ROWS = B * N            # 8192

_CACHE = {}


def _t3(a):
    """(M, N) -> (128, M/128, N) partition-tiled layout."""
    m, n = a.shape
    assert m % 128 == 0
    return np.ascontiguousarray(
        a.reshape(m // 128, 128, n).transpose(1, 0, 2)).astype(np.float32)


def _from3(a):
    p, m, n = a.shape
    return np.ascontiguousarray(a.transpose(1, 0, 2)).reshape(m * 128, n)


def _consts():
    if "dft" in _CACHE:
        return _CACHE["dft"]
    l = np.arange(N, dtype=np.float64)[:, None]
    k = np.arange(KP, dtype=np.float64)[None, :]
    mask = (k < KH).astype(np.float64)
    ang = 2.0 * np.pi * l * k / M2
    cr = np.cos(ang) * mask
    ci = -np.sin(ang) * mask
    dft_cri = np.concatenate([cr, ci], axis=1)            # (2048, 4352)
    w = np.where((k[0] == 0) | (k[0] == M2 // 2), 1.0, 2.0) * mask[0]
    kk = np.arange(KP, dtype=np.float64)[:, None]
    t = np.arange(N, dtype=np.float64)[None, :]
    ang2 = 2.0 * np.pi * kk * t / M2
    icos = (w[:, None] / M2) * np.cos(ang2)               # (2176, 2048)
    isin = (-w[:, None] / M2) * np.sin(ang2)
    idft_cs = np.concatenate([icos, isin], axis=0)        # (4352, 2048)
    decay = GAMMA ** np.arange(N, dtype=np.float64)       # lag 0 -> 1.0
    decay_t = decay.reshape(N // 128, 128).T              # (128, 16)
    _CACHE["dft"] = (_t3(dft_cri), _t3(idft_cs), decay_t.astype(np.float32))
    return _CACHE["dft"]


def _build():
    import concourse.bass as bass
    import concourse.mybir as mybir
    import concourse.tile as tile
    from concourse import bacc
    from concourse.kernels.tile_matmul import matmul_tile_kernel

    AFT = mybir.ActivationFunctionType
    ALU = mybir.AluOpType
    dt = mybir.dt.float32

    nc = bacc.Bacc(None, target_bir_lowering=False, debug=False, num_devices=8)

    def din(name, shape):
        return nc.dram_tensor(name, list(shape), dt, kind="ExternalInput")

    def dint(name, shape):
        return nc.dram_tensor(name, list(shape), dt)

    xTa = din("xTa", (128, KA // 128, ROWS))
    u_wa = din("u_wa", (128, KA // 128, DH))
    v_wa = din("v_wa", (128, KA // 128, DH))
    o_w = din("o_w", (128, DH // 128, E))
    p_aug = din("p_aug", (2, N))
    pw_aug = din("pw_aug", (2, R))
    lws = [din(f"lw{i}", (128, R // 128, R)) for i in range(3)]
    lbs = din("lbs", (128, 3 * (R // 128)))   # 3 layers x (128, 4)
    out_w = din("out_w", (128, R // 128, DH))
    outb = din("outb", (1, DH))
    decay = din("decay", (128, N // 128))
    dft = din("dft", (128, N // 128, 2 * KP))
    idft = din("idft", (128, 2 * KP // 128, N))
    out = nc.dram_tensor("out", [128, ROWS // 128, E], dt, kind="ExternalOutput")

    acoef = dint("acoef", (128, N // 128, DH))
    arai = dint("arai", (128, 2 * KP // 128, DH))
    xrxi = dint("xrxi", (128, B * 2 * KP // 128, DH))
    prpi = dint("prpi", (128, B * 2 * KP // 128, DH))
    uT = dint("uT", (128, DH // 128, ROWS))
    v = dint("v", (128, ROWS // 128, DH))
    tvT = dint("tvT", (128, DH // 128, ROWS))
    gT = dint("gT", (128, DH // 128, ROWS))

    KG = KP // 128            # 17 freq groups
    FG = R // 128             # 4 feature groups

    def silu_evict(nc_, psum, sbuf):
        nc_.scalar.activation(sbuf, psum, AFT.Silu)

    with tile.TileContext(nc) as tc:
        # ---------------- RPE MLP (feature-major, fully in SBUF) --------
        with (tc.tile_pool(name="mlp", bufs=1) as mp,
              tc.tile_pool(name="mlp_ps", bufs=2, space="PSUM") as mps):
            ones_col = mp.tile([128, 1], dt)      # K=128 -> M=1 reducer
            nc.vector.memset(ones_col[:], 1.0)
            one_row = mp.tile([1, 128], dt)       # K=1 -> 128-partition bcast
            nc.vector.memset(one_row[:], 1.0)
            c_sc = mp.tile([1, 1], dt)
            nc.vector.memset(c_sc[:], float(R ** -0.5))
            eps_sc = mp.tile([1, 1], dt)
            nc.vector.memset(eps_sc[:], EPS)

            pa_sb = mp.tile([2, N], dt)
            pw_sb = mp.tile([2, R], dt)
            lb_sb = mp.tile([128, 3 * FG], dt)
            nc.sync.dma_start(pa_sb[:], p_aug[:])
            nc.sync.dma_start(pw_sb[:], pw_aug[:])
            nc.sync.dma_start(lb_sb[:], lbs[:])

            h = [mp.tile([128, N], dt, name=f"h{g}", tag=f"h{g}") for g in range(FG)]
            # h0 = pos_idx @ pos_w + pos_b   (K=2), feature-major (512, 2048)
            for g in range(FG):
                for nch in range(N // 512):
                    ps = mps.tile([128, 512], dt, name="mmps", tag="mm")
                    nc.tensor.matmul(
                        ps[:], pw_sb[:, g * 128:(g + 1) * 128],
                        pa_sb[:, nch * 512:(nch + 1) * 512],
                        start=True, stop=True)
                    nc.vector.tensor_copy(h[g][:, nch * 512:(nch + 1) * 512], ps[:])

            def srms_relu(h_in, phi_out):
                # s[t] = sum_f h^2 ; factor = 1/(sqrt(s)/sqrt(R) + eps)
                sq = [mp.tile([128, N], dt, name=f"sq{g}", tag=f"sq{g}") for g in range(FG)]
                for g in range(FG):
                    nc.vector.tensor_mul(sq[g][:], h_in[g][:], h_in[g][:])
                fac = mp.tile([1, N], dt, name="fac", tag="fac")
                for nch in range(N // 512):
                    ps1 = mps.tile([1, 512], dt, name="redps", tag="red")
                    for g in range(FG):
                        nc.tensor.matmul(
                            ps1[:], ones_col[:],
                            sq[g][:, nch * 512:(nch + 1) * 512],
                            start=(g == 0), stop=(g == FG - 1))
                    sl = fac[:, nch * 512:(nch + 1) * 512]
                    nc.scalar.activation(sl, ps1[:], AFT.Sqrt)
                    nc.vector.tensor_scalar(
                        sl, sl, c_sc[:], eps_sc[:], ALU.mult, ALU.add)
                    nc.vector.reciprocal(sl, sl)
                fb = mp.tile([128, N], dt, name="fb", tag="fb")
                for nch in range(N // 512):
                    psb = mps.tile([128, 512], dt, name="bcps", tag="bc")
                    nc.tensor.matmul(
                        psb[:], one_row[:], fac[:, nch * 512:(nch + 1) * 512],
                        start=True, stop=True)
                    nc.vector.tensor_copy(fb[:, nch * 512:(nch + 1) * 512], psb[:])
                for g in range(FG):
                    nc.vector.tensor_mul(phi_out[g][:], h_in[g][:], fb[:])
                    nc.scalar.activation(phi_out[g][:], phi_out[g][:], AFT.Relu)

            phi = [mp.tile([128, N], dt, name=f"phi{g}", tag=f"phi{g}") for g in range(FG)]
            srms_relu(h, phi)

            lw_sb = mp.tile([128, FG, R], dt)
            for li in range(3):
                nc.sync.dma_start(lw_sb[:], lws[li][:])
                for g in range(FG):
                    for nch in range(N // 512):
                        ps = mps.tile([128, 512], dt, name="mmps", tag="mm")
                        for k in range(FG):
                            nc.tensor.matmul(
                                ps[:], lw_sb[:, k, g * 128:(g + 1) * 128],
                                phi[k][:, nch * 512:(nch + 1) * 512],
                                start=(k == 0), stop=(k == FG - 1))
                        sl = h[g][:, nch * 512:(nch + 1) * 512]
                        nc.vector.tensor_scalar(
                            sl, ps[:], lb_sb[:, li * FG + g:li * FG + g + 1],
                            None, ALU.add)
                srms_relu(h, phi)

            # coefs (t-major) = phi.T @ out_w  -> * decay + out_b -> acoef
            ow_sb = mp.tile([128, FG, DH], dt)
            ob_sb = mp.tile([1, DH], dt)
            dec_sb = mp.tile([128, N // 128], dt)
            nc.sync.dma_start(ow_sb[:], out_w[:])
            nc.sync.dma_start(ob_sb[:], outb[:])
            nc.sync.dma_start(dec_sb[:], decay[:])
            obb = mp.tile([128, DH], dt)
            psb = mps.tile([128, DH], dt, name="bc2ps", tag="bc")
            nc.tensor.matmul(psb[:], one_row[:], ob_sb[:], start=True, stop=True)
            nc.vector.tensor_copy(obb[:], psb[:])
            for m in range(N // 128):
                ps = mps.tile([128, DH], dt, name="mm2ps", tag="mm")
                for k in range(FG):
                    nc.tensor.matmul(
                        ps[:], phi[k][:, m * 128:(m + 1) * 128],
                        ow_sb[:, k, :], start=(k == 0), stop=(k == FG - 1))
                ac = mp.tile([128, DH], dt, name="ac", tag="ac")
                nc.vector.tensor_add(ac[:], ps[:], obb[:])
                nc.vector.tensor_scalar(
                    ac[:], ac[:], dec_sb[:, m:m + 1], None, ALU.mult)
                nc.sync.dma_start(acoef[:, m, :], ac[:])

        # ---------------- big matmuls via matmul_tile_kernel ------------
        # A: kernel spectrum  ArAi = dft.T @ acoef   (K=2048, M=4352, N=384)
        matmul_tile_kernel(tc, dft[:], acoef[:], arai[:])
        # B: uT = silu(u_wa.T @ xTa)                 (K=1152, M=384, N=8192)
        matmul_tile_kernel(tc, u_wa[:], xTa[:], uT[:], psum_evict_fn=silu_evict)
        # C: v = silu(xTa.T @ v_wa)                  (K=1152, M=8192, N=384)
        matmul_tile_kernel(tc, xTa[:], v_wa[:], v[:], psum_evict_fn=silu_evict)
        # D: forward DFT of v per batch
        for b in range(B):
            matmul_tile_kernel(
                tc, dft[:],
                v[:, b * (N // 128):(b + 1) * (N // 128), :],
                xrxi[:, b * 2 * KG:(b + 1) * 2 * KG, :])

        # E: pointwise complex multiply  P = A * X
        with (tc.tile_pool(name="pw", bufs=1) as pwp,
              tc.tile_pool(name="pw2", bufs=4) as pw2):
            ar_sb = pwp.tile([128, 2 * KG, DH], dt)
            nc.sync.dma_start(ar_sb[:], arai[:])
            for b in range(B):
                for g in range(KG):
                    xr = pw2.tile([128, DH], dt, name="xr", tag="xr")
                    xi = pw2.tile([128, DH], dt, name="xi", tag="xi")
                    nc.sync.dma_start(xr[:], xrxi[:, b * 2 * KG + g, :])
                    nc.sync.dma_start(xi[:], xrxi[:, b * 2 * KG + KG + g, :])
                    pr = pw2.tile([128, DH], dt, name="pr", tag="pr")
                    pi = pw2.tile([128, DH], dt, name="pi", tag="pi")
                    t1 = pw2.tile([128, DH], dt, name="t1", tag="t1")
                    nc.vector.tensor_mul(pr[:], ar_sb[:, g, :], xr[:])
                    nc.vector.tensor_mul(t1[:], ar_sb[:, KG + g, :], xi[:])
                    nc.vector.tensor_sub(pr[:], pr[:], t1[:])
                    nc.vector.tensor_mul(pi[:], ar_sb[:, g, :], xi[:])
                    nc.vector.tensor_mul(t1[:], ar_sb[:, KG + g, :], xr[:])
                    nc.vector.tensor_add(pi[:], pi[:], t1[:])
                    nc.sync.dma_start(prpi[:, b * 2 * KG + g, :], pr[:])
                    nc.sync.dma_start(prpi[:, b * 2 * KG + KG + g, :], pi[:])

        # F: inverse DFT  tvT_b = PrPi_b.T @ idft_cs  (K=4352, M=384, N=2048)
        for b in range(B):
            matmul_tile_kernel(
                tc, prpi[:, b * 2 * KG:(b + 1) * 2 * KG, :], idft[:],
                tvT[:, :, b * N:(b + 1) * N])

        # G: gate  gT = uT * tvT
        with tc.tile_pool(name="gate", bufs=4) as gp:
            for m in range(DH // 128):
                for nch in range(ROWS // 2048):
                    ut = gp.tile([128, 2048], dt, name="ut", tag="ut")
                    tt = gp.tile([128, 2048], dt, name="tt", tag="tt")
                    nc.sync.dma_start(ut[:], uT[:, m, nch * 2048:(nch + 1) * 2048])
                    nc.sync.dma_start(tt[:], tvT[:, m, nch * 2048:(nch + 1) * 2048])
                    nc.vector.tensor_mul(ut[:], ut[:], tt[:])
                    nc.sync.dma_start(gT[:, m, nch * 2048:(nch + 1) * 2048], ut[:])

        # H: partial o-projection  out = gT.T @ o_w  (K=384, M=8192, N=1024)
        matmul_tile_kernel(tc, gT[:], o_w[:], out[:])

    nc.compile()
    return nc


def _get_nc():
    if "nc" not in _CACHE:
        _CACHE["nc"] = _build()
    return _CACHE["nc"]


def kernel(x, u_w, u_b, v_w, v_b, o_w, o_b,
           pos_w, pos_b, lw0, lb0, lw1, lb1, lw2, lb2, out_w, out_b):
    from concourse.bass_utils import run_bass_kernel_spmd

    dft3, idft3, decay_t = _consts()
    x_flat = np.asarray(x, np.float32).reshape(ROWS, E)
    xTa = np.zeros((KA, ROWS), np.float32)
    xTa[:E] = x_flat.T
    xTa[E] = 1.0
    xTa3 = _t3(xTa)

    p_aug = np.stack([np.arange(N, dtype=np.float32),
                      np.ones(N, np.float32)])
    pw_aug = np.concatenate([pos_w, pos_b[None, :]], 0).astype(np.float32)
    # lbs layout: [:, li*4 + g] = lb_li[g*128 + p]
    lbs = np.concatenate(
        [lb.reshape(R // 128, 128).T for lb in (lb0, lb1, lb2)],
        axis=1).astype(np.float32)

    in_maps = []
    for h in range(H):
        sl = slice(h * DH, (h + 1) * DH)
        u_wa = np.zeros((KA, DH), np.float32)
        u_wa[:E] = u_w[:, sl]
        u_wa[E] = u_b[sl]
        v_wa = np.zeros((KA, DH), np.float32)
        v_wa[:E] = v_w[:, sl]
        v_wa[E] = v_b[sl]
        in_maps.append(dict(
            xTa=xTa3, u_wa=_t3(u_wa), v_wa=_t3(v_wa),
            o_w=_t3(np.ascontiguousarray(o_w[sl, :]).astype(np.float32)),
            p_aug=p_aug, pw_aug=pw_aug,
            lw0=_t3(lw0.astype(np.float32)), lw1=_t3(lw1.astype(np.float32)),
            lw2=_t3(lw2.astype(np.float32)), lbs=lbs,
            out_w=_t3(np.ascontiguousarray(out_w[:, sl]).astype(np.float32)),
            outb=np.ascontiguousarray(out_b[None, sl]).astype(np.float32),
            decay=decay_t, dft=dft3, idft=idft3,
        ))

    nc = _get_nc()
    res = run_bass_kernel_spmd(nc, in_maps, core_ids=list(range(8)),
                               trace=bool(_CACHE.get("trace")))
    _CACHE["last_res"] = res
    acc = np.zeros((ROWS, E), np.float32)
    for i in range(H):
        acc += _from3(res.results[i]["out"])
    acc += o_b[None, :]
    return acc.reshape(B, N, E)



# revision 26
# speedup vs baseline: 1.6427x; 1.6427x over previous
"""GTU (gated Toeplitz unit) Bass kernel for 8 TRN2 NeuronCores.

Sharding: tensor-parallel over heads (H=8 -> 1 head/core). Each core runs a
fully fused bf16 pipeline:
  R: RPE MLP -> Toeplitz coefs (t-major, SBUF)
  P: u/v projections (one pass over x^T, silu, v kept in SBUF, u spilled)
  F: forward real-DFT of [v(4 batches) | coefs] as one tiled GEMM with the
     A*V complex pointwise multiply fused into the PSUM eviction
  I: inverse DFT (idft stationary, spectra moving) with the u-gate fused
     into the eviction
  T: PE-transpose of the gated tensor to d-major
  O: partial o-projection -> bf16 partial output; host sums partials + bias.
All heavy matmuls run in bf16 (4x the fp32 PE rate); f32r is used for the
tiny fp32 RPE helper matmuls.
"""

import numpy as np

B, N, E = 4, 2048, 1024
H = 8
D1 = 3 * E
DH = D1 // H            # 384
R = 512
GAMMA = 0.99
EPS = 1e-8
M2 = 2 * N              # 4096 circular conv length
KH = M2 // 2 + 1        # 2049 rfft bins
KP = 2176               # bins padded to 17*128
KG = KP // 128          # 17 freq tile-groups (re); total re+im = 34
KA = 1024 + 128         # augmented contraction for x (bias row)
ROWS = B * N            # 8192

_CACHE = {}


def _bf16(a):
    import ml_dtypes
    return np.ascontiguousarray(np.asarray(a)).astype(ml_dtypes.bfloat16)


def _consts():
    if "c" in _CACHE:
        return _CACHE["c"]
    l = np.arange(N, dtype=np.float64)[:, None]
    k = np.arange(KP, dtype=np.float64)[None, :]
    mask = (k < KH).astype(np.float64)
    ang = 2.0 * np.pi * l * k / M2
    cr = np.cos(ang) * mask
    ci = -np.sin(ang) * mask
    dft_cri = np.concatenate([cr, ci], axis=1)            # (2048, 4352)
    w = np.where((k[0] == 0) | (k[0] == M2 // 2), 1.0, 2.0) * mask[0]
    kk = np.arange(KP, dtype=np.float64)[:, None]
    t = np.arange(N, dtype=np.float64)[None, :]
    ang2 = 2.0 * np.pi * kk * t / M2
    icos = (w[:, None] / M2) * np.cos(ang2)               # (2176, 2048)
    isin = (-w[:, None] / M2) * np.sin(ang2)
    idft_cs = np.concatenate([icos, isin], axis=0)        # (4352, 2048)

    # stationary-tile-major layouts: one DMA per m/tb tile group
    dft_r = _bf16(dft_cri.reshape(16, 128, 34, 128).transpose(1, 2, 0, 3))
    idft_r = _bf16(idft_cs.reshape(34, 128, 16, 128).transpose(1, 2, 0, 3))
    decay = GAMMA ** np.arange(N, dtype=np.float64)
    decay_t = np.ascontiguousarray(
        decay.reshape(N // 128, 128).T).astype(np.float32)   # (128, 16)
    pa = np.stack([np.arange(N, dtype=np.float32), np.ones(N, np.float32)])
    _CACHE["c"] = (dft_r, idft_r, decay_t, pa)
    return _CACHE["c"]


def _build():
    import concourse.mybir as mybir
    import concourse.tile as tile
    from concourse import bacc
    from concourse.masks import make_identity

    AFT = mybir.ActivationFunctionType
    ALU = mybir.AluOpType
    f32 = mybir.dt.float32
    f32r = mybir.dt.float32r
    bf = mybir.dt.bfloat16

    nc = bacc.Bacc(None, target_bir_lowering=False, debug=False, num_devices=8)

    def din(name, shape, dt=bf):
        return nc.dram_tensor(name, list(shape), dt, kind="ExternalInput")

    # ---- DRAM tensors ----
    xTa_r = din("xTa_r", (128, 64, 9, 128))
    uvw = din("uvw", (128, 9, 768))
    o_w3 = din("o_w3", (128, 3, 1024))
    dft = din("dft", (128, 34, 16, 128))
    idft = din("idft", (128, 16, 34, 128))
    lws = din("lws", (128, 12, R))
    owr = din("owr", (128, 4, DH))
    pa_d = din("pa", (2, N), f32)
    pwc_d = din("pwc", (128, 8), f32)   # [pos_w cols (4) | pos_b cols (4)]
    lbs_d = din("lbs", (128, 12), f32)
    obb_d = din("obb", (128, DH), f32)
    dec_d = din("decay", (128, 16), f32)
    u_sp = nc.dram_tensor("u_sp", [128, 16, 4, DH], bf)       # u spill
    out = nc.dram_tensor("out", [128, 64, 1024], bf, kind="ExternalOutput")

    FG = R // 128   # 4

    with tile.TileContext(nc) as tc:
        acv_ctx = tc.tile_pool(name="acv", bufs=1)
        acv = acv_ctx.__enter__()
        acoef = acv.tile([128, 16, DH], bf)      # Toeplitz coefs, t-major
        v_t = acv.tile([128, 16, 4 * DH], bf)    # v, t-major, batches adjacent

        # ================= phase R: RPE MLP =================
        with (tc.tile_pool(name="rpe", bufs=1) as rp,
              tc.tile_pool(name="rpe2", bufs=2) as rp2,
              tc.tile_pool(name="rpe_ps", bufs=2, space="PSUM") as rps):
            pa_sb = rp.tile([2, N], f32)
            pwc_sb = rp.tile([128, 8], f32)
            lbs_sb = rp.tile([128, 12], f32)
            lws_sb = rp.tile([128, 12, R], bf)
            owr_sb = rp.tile([128, FG, DH], bf)
            obb_sb = rp.tile([128, DH], f32)
            dec_sb = rp.tile([128, 16], f32)
            nc.sync.dma_start(pa_sb[:], pa_d[:])
            nc.sync.dma_start(pwc_sb[:], pwc_d[:])
            nc.sync.dma_start(lbs_sb[:], lbs_d[:])
            nc.sync.dma_start(lws_sb[:], lws[:])
            nc.sync.dma_start(owr_sb[:], owr[:])
            nc.sync.dma_start(obb_sb[:], obb_d[:])
            nc.sync.dma_start(dec_sb[:], dec_d[:])
            ones_col = rp.tile([128, 1], bf)
            nc.vector.memset(ones_col[:], 1.0)
            c_sc = rp.tile([1, 1], f32)
            nc.vector.memset(c_sc[:], float(R ** -0.5))
            eps_sc = rp.tile([1, 1], f32)
            nc.vector.memset(eps_sc[:], EPS)

            h = [rp.tile([128, N], f32, name=f"h{g}", tag=f"h{g}")
                 for g in range(FG)]
            phi = [rp.tile([128, N], bf, name=f"phi{g}", tag=f"phi{g}")
                   for g in range(FG)]
            fac = rp.tile([1, N], f32)
            fb = rp.tile([128, N], f32)

            # first layer: h[f, t] = pos[t] * pos_w[f] + pos_b[f]  (exact,
            # per-partition scalars on DVE after a pos broadcast)
            pb = rp.tile([128, N], f32)
            nc.gpsimd.partition_broadcast(pb[:], pa_sb[0:1, :])
            for g in range(FG):
                nc.vector.tensor_scalar(
                    h[g][:], pb[:], pwc_sb[:, g:g + 1],
                    pwc_sb[:, 4 + g:4 + g + 1], ALU.mult, ALU.add)

            def srms_relu():
                # fac[t] = 1 / (sqrt(sum_f h^2) / sqrt(R) + eps); phi=relu(h*fac)
                for nch in range(N // 512):
                    ps1 = rps.tile([1, 512], f32, name="redps", tag="red")
                    for g in range(FG):
                        sq = rp2.tile([128, 512], bf, name="sq", tag="sq")
                        sl = slice(nch * 512, (nch + 1) * 512)
                        nc.vector.tensor_mul(sq[:], h[g][:, sl], h[g][:, sl])
                        nc.tensor.matmul(
                            ps1[:], ones_col[:], sq[:],
                            start=(g == 0), stop=(g == FG - 1))
                    nc.scalar.activation(
                        fac[:, nch * 512:(nch + 1) * 512], ps1[:], AFT.Sqrt)
                nc.vector.tensor_scalar(
                    fac[:], fac[:], c_sc[:], eps_sc[:], ALU.mult, ALU.add)
                nc.vector.reciprocal(fac[:], fac[:])
                nc.gpsimd.partition_broadcast(fb[:], fac[:])
                for g in range(FG):
                    nc.vector.tensor_mul(phi[g][:], h[g][:], fb[:])
                    nc.scalar.activation(phi[g][:], phi[g][:], AFT.Relu)

            srms_relu()
            for li in range(3):
                for g in range(FG):
                    for nch in range(N // 512):
                        ps = rps.tile([128, 512], f32, name="mmps", tag="mm")
                        for kk in range(FG):
                            nc.tensor.matmul(
                                ps[:],
                                lws_sb[:, li * FG + kk, g * 128:(g + 1) * 128],
                                phi[kk][:, nch * 512:(nch + 1) * 512],
                                start=(kk == 0), stop=(kk == FG - 1))
                        nc.scalar.activation(
                            h[g][:, nch * 512:(nch + 1) * 512], ps[:],
                            AFT.Identity,
                            bias=lbs_sb[:, li * FG + g:li * FG + g + 1])
                srms_relu()

            # out proj: acoef[t, d] = (phi(t)^T @ out_w + out_b) * decay(t)
            for m in range(16):
                ps = rps.tile([128, DH], f32, name="ops", tag="mm")
                for kk in range(FG):
                    nc.tensor.matmul(
                        ps[:], phi[kk][:, m * 128:(m + 1) * 128],
                        owr_sb[:, kk, :], start=(kk == 0), stop=(kk == FG - 1))
                tmp = rp2.tile([128, DH], f32, name="actmp", tag="actmp")
                nc.vector.tensor_add(tmp[:], ps[:], obb_sb[:])
                nc.vector.tensor_scalar(
                    acoef[:, m, :], tmp[:], dec_sb[:, m:m + 1], None, ALU.mult)

        # ================= phase P: u/v projections =================
        with (tc.tile_pool(name="pP", bufs=3) as pp,
              tc.tile_pool(name="pPw", bufs=1) as ppw,
              tc.tile_pool(name="pP_ps", bufs=2, space="PSUM") as pps):
            uvw_sb = ppw.tile([128, 9, 768], bf)
            nc.sync.dma_start(uvw_sb[:], uvw[:])
            for m in range(64):
                xt = pp.tile([128, 9, 128], bf, name="xt", tag="xt")
                nc.sync.dma_start(xt[:], xTa_r[:, m])
                ps_u = pps.tile([128, DH], f32, name="psu", tag="psu")
                ps_v = pps.tile([128, DH], f32, name="psv", tag="psv")
                for kk in range(9):
                    nc.tensor.matmul(ps_u[:], xt[:, kk], uvw_sb[:, kk, 0:DH],
                                     start=(kk == 0), stop=(kk == 8))
                    nc.tensor.matmul(ps_v[:], xt[:, kk], uvw_sb[:, kk, DH:768],
                                     start=(kk == 0), stop=(kk == 8))
                ust = pp.tile([128, DH], bf, name="ust", tag="ust")
                nc.scalar.activation(ust[:], ps_u[:], AFT.Silu)
                nc.sync.dma_start(u_sp[:, m % 16, m // 16, :], ust[:])
                b_, tg_ = m // 16, m % 16
                nc.scalar.activation(
                    v_t[:, tg_, b_ * DH:(b_ + 1) * DH], ps_v[:], AFT.Silu)

        # ============ phase F: forward DFT + pointwise multiply ============
        prpi_ctx = tc.tile_pool(name="prpi", bufs=1, side="right")
        prp = prpi_ctx.__enter__()
        prpi = prp.tile([128, 34, 4 * DH], bf)   # spectra products (re, im)

        with (tc.tile_pool(name="pF", bufs=2) as pf,
              tc.tile_pool(name="pF2", bufs=2) as pf2,
              tc.tile_pool(name="pF_ps", bufs=2, space="PSUM") as fps):
            sc_re = None
            for g in range(KG):
                for half in range(2):
                    m = g + 17 * half
                    dt_sb = pf.tile([128, 16, 128], bf, name="dt", tag="dt")
                    nc.sync.dma_start(dt_sb[:], dft[:, m])
                    ps_v = fps.tile([128, 4 * DH], f32, name="fv", tag="fv")
                    ps_a = fps.tile([128, DH], f32, name="fa", tag="fa")
                    for kk in range(16):
                        lhs = dt_sb[:, kk]
                        st = kk == 0
                        sp = kk == 15
                        for jj in range(3):
                            nc.tensor.matmul(
                                ps_v[:, jj * 512:(jj + 1) * 512], lhs,
                                v_t[:, kk, jj * 512:(jj + 1) * 512],
                                start=st, stop=sp)
                        nc.tensor.matmul(
                            ps_a[:], lhs, acoef[:, kk, :], start=st, stop=sp)
                    sc = pf2.tile([128, 5, DH], bf, name="sc",
                                  tag="scR" if half == 0 else "scI")
                    nc.scalar.activation(
                        sc[:, 0:4, :].rearrange("p a b -> p (a b)"),
                        ps_v[:], AFT.Copy)
                    nc.scalar.activation(sc[:, 4], ps_a[:], AFT.Copy)
                    if half == 0:
                        sc_re = sc
                    else:
                        # complex multiply: P = A * X for 4 batches at once
                        ar = sc_re[:, 4:5, :].broadcast_to([128, 4, DH])
                        ai = sc[:, 4:5, :].broadcast_to([128, 4, DH])
                        xr = sc_re[:, 0:4, :]
                        xi = sc[:, 0:4, :]
                        t1 = pf2.tile([128, 4, DH], bf, name="t1", tag="t1")
                        t2 = pf2.tile([128, 4, DH], bf, name="t2", tag="t2")
                        pr = prpi[:, g, :].rearrange("p (a b) -> p a b", a=4)
                        pi = prpi[:, g + 17, :].rearrange("p (a b) -> p a b", a=4)
                        nc.vector.tensor_mul(t1[:], ar, xr)
                        nc.vector.tensor_mul(t2[:], ai, xi)
                        nc.vector.tensor_sub(pr, t1[:], t2[:])
                        nc.vector.tensor_mul(t1[:], ar, xi)
                        nc.vector.tensor_mul(t2[:], ai, xr)
                        nc.vector.tensor_add(pi, t1[:], t2[:])

        acv_ctx.__exit__(None, None, None)

        gt_ctx = tc.tile_pool(name="gt", bufs=1)
        gtp = gt_ctx.__enter__()
        g_t = gtp.tile([128, 16, 4 * DH], bf)    # gated tv, t-major

        # ============ phase I: inverse DFT + gate ============
        with (tc.tile_pool(name="pI", bufs=2) as pi_pool,
              tc.tile_pool(name="pI_ps", bufs=2, space="PSUM") as ips):
            for tb in range(16):
                id_sb = pi_pool.tile([128, 34, 128], bf, name="idt", tag="idt")
                nc.sync.dma_start(id_sb[:], idft[:, tb])
                u_sl = pi_pool.tile([128, 4, DH], bf, name="usl", tag="usl")
                nc.sync.dma_start(u_sl[:], u_sp[:, tb])
                ps = ips.tile([128, 4 * DH], f32, name="ips", tag="ips")
                for kk in range(34):
                    lhs = id_sb[:, kk]
                    rhs = prpi[:, kk, :]
                    st = kk == 0
                    sp = kk == 33
                    for jj in range(3):
                        nc.tensor.matmul(
                            ps[:, jj * 512:(jj + 1) * 512], lhs,
                            rhs[:, jj * 512:(jj + 1) * 512],
                            start=st, stop=sp)
                nc.vector.tensor_mul(
                    g_t[:, tb, :], ps[:],
                    u_sl[:].rearrange("p a b -> p (a b)"))

        prpi_ctx.__exit__(None, None, None)

        gT_ctx = tc.tile_pool(name="gT", bufs=1)
        gTp = gT_ctx.__enter__()
        gT = gTp.tile([128, 3, ROWS], bf)        # gated tv, d-major
        ow_sb = gTp.tile([128, 3, 1024], bf)
        ident = gTp.tile([128, 128], bf)
        nc.sync.dma_start(ow_sb[:], o_w3[:])
        make_identity(nc, ident)

        # ============ phase T: transpose gate to d-major ============
        with tc.tile_pool(name="pT_ps", bufs=4, space="PSUM") as tps:
            i = 0
            for tb in range(16):
                for b in range(4):
                    for dg in range(3):
                        pt = tps.tile([128, 128], bf, name="pt", tag="pt")
                        nc.tensor.transpose(
                            pt[:],
                            g_t[:, tb, b * DH + dg * 128: b * DH + (dg + 1) * 128],
                            ident[:])
                        dst = gT[:, dg, b * 2048 + tb * 128:
                                 b * 2048 + (tb + 1) * 128]
                        if i % 2 == 0:
                            nc.vector.tensor_copy(dst, pt[:])
                        else:
                            nc.scalar.activation(dst, pt[:], AFT.Copy)
                        i += 1

        # ============ phase O: partial o-projection ============
        with (tc.tile_pool(name="pO", bufs=3) as po,
              tc.tile_pool(name="pO_ps", bufs=2, space="PSUM") as ops):
            for m in range(64):
                ps0 = ops.tile([128, 512], f32, name="o0", tag="o0")
                ps1 = ops.tile([128, 512], f32, name="o1", tag="o1")
                for kk in range(3):
                    lhs = gT[:, kk, m * 128:(m + 1) * 128]
                    nc.tensor.matmul(ps0[:], lhs, ow_sb[:, kk, 0:512],
                                     start=(kk == 0), stop=(kk == 2))
                    nc.tensor.matmul(ps1[:], lhs, ow_sb[:, kk, 512:1024],
                                     start=(kk == 0), stop=(kk == 2))
                ost = po.tile([128, 1024], bf, name="ost", tag="ost")
                nc.scalar.activation(ost[:, 0:512], ps0[:], AFT.Copy)
                nc.scalar.activation(ost[:, 512:1024], ps1[:], AFT.Copy)
                nc.sync.dma_start(out[:, m, :], ost[:])

        gT_ctx.__exit__(None, None, None)
        gt_ctx.__exit__(None, None, None)

    nc.compile()
    return nc


def _get_nc():
    if "nc" not in _CACHE:
        _CACHE["nc"] = _build()
    return _CACHE["nc"]


def _prep_inputs(x, u_w, u_b, v_w, v_b, o_w, pos_w, pos_b,
                 lw0, lb0, lw1, lb1, lw2, lb2, out_w, out_b):
    dft_r, idft_r, decay_t, pa = _consts()

    x_flat = np.asarray(x, np.float32).reshape(ROWS, E)
    xTa = np.zeros((KA, ROWS), np.float32)
    xTa[:E] = x_flat.T
    xTa[E] = 1.0
    xTa_r = _bf16(xTa.reshape(9, 128, 64, 128).transpose(1, 2, 0, 3))

    pwc = np.concatenate(
        [np.asarray(pos_w, np.float32).reshape(4, 128).T,
         np.asarray(pos_b, np.float32).reshape(4, 128).T], axis=1)
    pwc = np.ascontiguousarray(pwc)
    lbs = np.concatenate(
        [lb.reshape(R // 128, 128).T for lb in (lb0, lb1, lb2)],
        axis=1).astype(np.float32)
    lws_bf = _bf16(np.concatenate(
        [lw.reshape(4, 128, R) for lw in (lw0, lw1, lw2)],
        axis=0).transpose(1, 0, 2))

    in_maps = []
    for hh in range(H):
        sl = slice(hh * DH, (hh + 1) * DH)
        w = np.zeros((KA, 768), np.float32)
        w[:E, :DH] = u_w[:, sl]
        w[:E, DH:] = v_w[:, sl]
        w[E, :DH] = u_b[sl]
        w[E, DH:] = v_b[sl]
        uvw = _bf16(w.reshape(9, 128, 768).transpose(1, 0, 2))
        o_w3 = _bf16(np.asarray(o_w[sl, :]).reshape(3, 128, 1024)
                     .transpose(1, 0, 2))
        owr = _bf16(np.asarray(out_w[:, sl]).reshape(4, 128, DH)
                    .transpose(1, 0, 2))
        obb = np.ascontiguousarray(
            np.broadcast_to(np.asarray(out_b[sl], np.float32), (128, DH)))
        in_maps.append(dict(
            xTa_r=xTa_r, uvw=uvw, o_w3=o_w3, dft=dft_r, idft=idft_r,
            lws=lws_bf, owr=owr, pa=pa, pwc=pwc, lbs=lbs, obb=obb,
            decay=decay_t,
        ))
    return in_maps


def kernel(x, u_w, u_b, v_w, v_b, o_w, o_b,
           pos_w, pos_b, lw0, lb0, lw1, lb1, lw2, lb2, out_w, out_b):
    from concourse.bass_utils import run_bass_kernel_spmd

    in_maps = _prep_inputs(x, u_w, u_b, v_w, v_b, o_w, pos_w, pos_b,
                           lw0, lb0, lw1, lb1, lw2, lb2, out_w, out_b)
    nc = _get_nc()
    res = run_bass_kernel_spmd(nc, in_maps, core_ids=list(range(8)),
                               trace=bool(_CACHE.get("trace")))
    _CACHE["last_res"] = res
    acc = np.zeros((ROWS, E), np.float32)
    for i in range(H):
        o = res.results[i]["out"].astype(np.float32)   # (128, 64, 1024)
        acc += o.transpose(1, 0, 2).reshape(ROWS, E)
    acc += np.asarray(o_b, np.float32)[None, :]
    return acc.reshape(B, N, E)
